# revision 32
# baseline (speedup 1.0000x reference)
"""Trainium2 Bass kernel for OldNeighborhoodEncoder (segment_reduce).

Math (reference):
    fc1    = relu(X @ W1.T + b1)            # [N, 64], X = [N, 3]
    pooled = segment_max(fc1, cluster, S)   # [S, 64], cluster = arange(N)//32
    h      = relu(pooled @ W1g.T + b1g)     # [S, 64]
    out    = relu(h @ W2g.T + b2g)          # [S, 128]

Hardcoded sizes: N=1048576, S=32768 (32 pts/cluster), FEATURE=64, FG0=64,
FG1=128, 8 cores. Data-parallel over points: core d handles points
[d*131072, (d+1)*131072) == clusters [d*4096, (d+1)*4096); no collectives.

Device layout (per core):
  xt [6, 65536]: col c = 512*g + o (g in 0..127, o in 0..511); rows 0-2 =
    xyz of point 1024*g + o, rows 3-5 = xyz of point 1024*g + 512 + o.
  wpack [6,128] = blockdiag(W1.T, W1.T): one matmul column-block computes
    fc1 (pre-bias) for TWO 512-point chunks at once -> full 128-partition
    PE output. Bias+relu are deferred past the max (monotone).
  psum [128,4,16,32]: bank b holds g = 4i+b; view [.., q, t] with o=32q+t,
    so a single DVE reduce over t pools 4*16 = 64 cluster-halves.
  pooled [128, 32, 4, 16]: pooled[64a+f, i, b, q] = max_z of cluster
    128i + 32b + 16a + q, feature f.
  Tail: relu(+b1) -> blockdiag(W1g.T) matmul -> relu(+b1g) ->
    W2g.T matmul (K=64, separately for a=0 from partitions 0:64 and a=1
    from 64:128) -> relu(+b2g) -> outA/outB [128, 2048].

v1.5 perf structure: the main loop is DVE-reduce-bound (Pool/GPSIMD has no
legal max op on this target, so DVE does all 32 chunk reductions); weight
DMAs go on the Scalar queue (HWDGE; gpsimd SWDGE blocked the first matmul
~7us); relu(+b1) of pooled happens in slices during the main loop on ACT;
the tail MLP is pipelined in 512-col sub-slices with relu work split
between ACT and DVE, and output DMAs are split in halves on two queues.
"""

import sys
import numpy as np

if "/opt/trn_rl_repo" not in sys.path:
    sys.path.insert(0, "/opt/trn_rl_repo")

N = 1048576
S = 32768
PTS_PER_CLUSTER = 32
FEATURE = 64
FG0 = 64
FG1 = 128
NCORES = 8
NPC = N // NCORES          # 131072 points per core
SPC = S // NCORES          # 4096 clusters per core
G = NPC // 1024            # 128 column-groups of 512
NCHUNK = 32                # psum chunks per core (each = 4 groups)

_PROGRAM = None  # (nc, input_names) cache


def _build_program():
    from concourse import bacc, bass, tile

    mybir = bass.mybir
    f32 = mybir.dt.float32
    # bf16 matmul path: full-rate 1 cycle/row on PE (f32r measured ~3x
    # slower on hw despite the cost model's claim), halves the xt DMA.
    fmm = mybir.dt.bfloat16
    AX = mybir.AxisListType

    nc = bacc.Bacc("TRN2", target_bir_lowering=False, debug=False)

    xt = nc.dram_tensor("xt", [6, G * 512], fmm, kind="ExternalInput").ap()
    wpack = nc.dram_tensor("wpack", [6, 128], fmm, kind="ExternalInput").ap()
    b1d = nc.dram_tensor("b1d", [128, 1], f32, kind="ExternalInput").ap()
    w1gs = nc.dram_tensor("w1gs", [128, 64], fmm, kind="ExternalInput").ap()
    b1gd = nc.dram_tensor("b1gd", [128, 1], f32, kind="ExternalInput").ap()
    w2gt = nc.dram_tensor("w2gt", [128, 128], fmm, kind="ExternalInput").ap()
    b2g = nc.dram_tensor("b2g", [128, 1], f32, kind="ExternalInput").ap()
    outA = nc.dram_tensor("outA", [128, 2048], f32, kind="ExternalOutput").ap()
    outB = nc.dram_tensor("outB", [128, 2048], f32, kind="ExternalOutput").ap()

    # chunks whose pooling runs as a direct f32 DVE reduce from PSUM; the
    # rest are relu(+b1)-copied PSUM->SBUF bf16 by ACT, then max-pooled on
    # DVE with a tensor_tensor tree (bf16 2x_1p: 2 results/cycle), four
    # chunks per tree pass to amortize the ~95ns/op DVE overhead.
    D_CHUNKS = (3, 8, 13, 18, 23, 28)
    d_index = {k: i for i, k in enumerate(D_CHUNKS)}

    Relu = mybir.ActivationFunctionType.Relu
    add = mybir.AluOpType.add
    vmax = mybir.AluOpType.max

    with tile.TileContext(nc) as tc:
        with (
            tc.tile_pool(name="w", bufs=1) as wp,
            tc.tile_pool(name="x", bufs=3) as xp,
            tc.tile_pool(name="pre", bufs=3) as prep,
            tc.tile_pool(name="scr", bufs=2) as scrp,
            tc.tile_pool(name="acc", bufs=1) as accp,
            tc.tile_pool(name="ps", bufs=2, space=bass.MemorySpace.PSUM) as pp,
        ):
            # wpack lives at SBUF partitions 0:6 AND 32:38: consecutive
            # matmuls alternate PE row-groups q0/q1, which the 32x32
            # sub-array hardware runs CONCURRENTLY (2 matmuls per ~427ns
            # instead of ~512ns each -- measured).
            wpack_t = wp.tile([38, 128], fmm, tag="wpack")
            b1d_t = wp.tile([128, 1], f32, tag="b1d")
            w1gs_t = wp.tile([128, 64], fmm, tag="w1gs")
            b1gd_t = wp.tile([128, 1], f32, tag="b1gd")
            w2gt_t = wp.tile([128, 128], fmm, tag="w2gt")
            b2g_t = wp.tile([128, 1], f32, tag="b2g")
            # wpack on the Sync queue ahead of xt: it gates the first
            # matmul, and sync's queue drains it ~1us sooner than scalar's.
            nc.sync.dma_start(wpack_t[0:6, :], wpack[:])
            nc.sync.dma_start(wpack_t[32:38, :], wpack[:])
            for t, d in (
                (b1d_t, b1d),
                (w1gs_t, w1gs),
                (b1gd_t, b1gd),
                (w2gt_t, w2gt),
                (b2g_t, b2g),
            ):
                nc.scalar.dma_start(t[:], d[:])

            pooledF = accp.tile([128, len(D_CHUNKS), 64], f32, tag="pooledF")
            pooledR = accp.tile([128, 2048], fmm, tag="pooledR")
            pooledRv = pooledR[:].rearrange("p (c e) -> p c e", e=64)

            def pair_tree_ops(pt, sc, ks):
                # 5-level pairwise-max tree over two chunks: [128, 2, 64, 32]
                # bf16 -> two pooledR slices [128, 64]. Returns (est_ns,
                # thunk) steps so the flush can be rate-controlled; levels
                # 1-4 run at 2 elem-results/cycle (bf16 2x_1p).
                l1 = sc[:, :, 0:1024].rearrange("p q (g e) -> p q g e", g=64)
                l2 = sc[:, :, 1024:1536].rearrange("p q (g e) -> p q g e", g=64)
                l3 = sc[:, :, 1536:1792].rearrange("p q (g e) -> p q g e", g=64)
                l4 = sc[:, :, 1792:1920].rearrange("p q (g e) -> p q g e", g=64)
                v = pt[:]
                ka, kb = ks
                outv = pooledRv[:, ka : kb + 1 : (kb - ka)]
                tt = nc.vector.tensor_tensor
                return [
                    (1170, lambda: tt(
                        l1, v[:, :, :, 0:16], v[:, :, :, 16:32], op=vmax)),
                    (630, lambda: tt(
                        l2, l1[:, :, :, 0:8], l1[:, :, :, 8:16], op=vmax)),
                    (370, lambda: tt(
                        l3, l2[:, :, :, 0:4], l2[:, :, :, 4:8], op=vmax)),
                    (230, lambda: tt(
                        l4, l3[:, :, :, 0:2], l3[:, :, :, 2:4], op=vmax)),
                    (230, lambda: tt(
                        outv,
                        l4[:, :, :, 0:1].rearrange("p q g e -> p q (g e)"),
                        l4[:, :, :, 1:2].rearrange("p q g e -> p q (g e)"),
                        op=vmax)),
                ]

            # main loop: fc1 matmuls + split segment-max pooling. Tree steps
            # are drip-fed into the DVE queue (~<=1.5us per chunk) so a
            # D-chunk's psum-freeing reduce is never stuck behind a long
            # burst of tree work.
            pair = None
            step_q = []
            xt_t = None
            for k in range(NCHUNK):
                if k % 4 == 0:
                    # xt tile: even 512-col blocks at partitions 0:6,
                    # odd blocks at 32:38 (for q0/q1 row-group alternation).
                    # Even halves on the sync queue, odd halves on the (idle)
                    # gpsimd queue — each HWDGE DMA costs ~730ns of queue
                    # issue time, so splitting queues halves the startup
                    # serialization; scalar stays clear for the ACT copies.
                    xt_t = xp.tile([38, 4096], fmm, tag="xt")
                    c0 = k * 2048
                    src = xt[:, c0 : c0 + 8192].rearrange(
                        "p (b c) -> p b c", c=1024)
                    dste = xt_t[0:6, :].rearrange("p (b c) -> p b c", c=512)
                    dsto = xt_t[32:38, :].rearrange("p (b c) -> p b c", c=512)
                    if k == 0:
                        # split so chunk 0's blocks land first
                        nc.sync.dma_start(dste[:, 0:2], src[:, 0:2, 0:512])
                        nc.gpsimd.dma_start(dsto[:, 0:2], src[:, 0:2, 512:1024])
                        nc.sync.dma_start(dste[:, 2:8], src[:, 2:8, 0:512])
                        nc.gpsimd.dma_start(dsto[:, 2:8], src[:, 2:8, 512:1024])
                    else:
                        nc.sync.dma_start(dste[:], src[:, :, 0:512])
                        nc.gpsimd.dma_start(dsto[:], src[:, :, 512:1024])
                ps = pp.tile([128, 4, 16, 32], f32, tag="ps")
                for b in range(4):
                    t = (k % 4) * 4 + b
                    home = 0 if t % 2 == 0 else 32
                    col = (t // 2) * 512
                    nc.tensor.matmul(
                        ps[:, b],
                        wpack_t[home : home + 6, :],
                        xt_t[home : home + 6, col : col + 512],
                    )
                if k in d_index:
                    di = d_index[k]
                    nc.vector.reduce_max(pooledF[:, di], ps[:], axis=AX.X)
                    budget = 420
                else:
                    if pair is None:
                        pt = prep.tile([128, 2, 64, 32], fmm, tag="pre")
                        sc = scrp.tile([128, 2, 1920], fmm, tag="scr")
                        pair = (pt, sc, [])
                    pt, sc, ks = pair
                    nc.scalar.activation(
                        pt[:, len(ks)], ps[:], Relu, bias=b1d_t[:])
                    ks.append(k)
                    if len(ks) == 2:
                        step_q.extend(pair_tree_ops(pt, sc, ks))
                        pair = None
                    budget = 1500
                while step_q and step_q[0][0] <= budget:
                    est, thunk = step_q.pop(0)
                    thunk()
                    budget -= est
            for _, thunk in step_q:
                thunk()
            # relu(+b1) all D-chunk pooled slices in one strided op
            # (D_CHUNKS is an arithmetic sequence, so the output AP is
            # regular: stride D_CHUNKS[1]-D_CHUNKS[0] chunks)
            dstep = D_CHUNKS[1] - D_CHUNKS[0]
            nc.vector.tensor_scalar(
                pooledRv[:, D_CHUNKS[0] : D_CHUNKS[-1] + 1 : dstep],
                pooledF[:], b1d_t[:], 0.0, op0=add, op1=vmax,
            )

            # tail MLP: h = relu(pooledR @ W1g.T + b1g), out = relu(h @ ...).
            # The h matmul splits blockdiag K=128 into two K=64 matmuls on
            # row-groups 0/64 which the PE runs concurrently.
            hps = pp.tile([128, 4, 16, 32], f32, tag="ps")
            hR = accp.tile([128, 2048], fmm, tag="hR")
            def evict(dst, src, bias_t, j):
                # j<3 evictions ride ACT (idle once the copies end) so the
                # DVE queue stays clear for the final pair-tree and the
                # latency-critical j==3 chain, which runs on DVE.
                if j == 3:
                    nc.vector.tensor_scalar(
                        dst, src, bias_t[:], 0.0, op0=add, op1=vmax)
                else:
                    nc.scalar.activation(dst, src, Relu, bias=bias_t[:])

            for j in range(4):
                nc.tensor.matmul(
                    hps[0:64, j],
                    w1gs_t[0:64, :],
                    pooledR[0:64, j * 512 : (j + 1) * 512],
                )
                nc.tensor.matmul(
                    hps[64:128, j],
                    w1gs_t[64:128, :],
                    pooledR[64:128, j * 512 : (j + 1) * 512],
                )
                evict(hR[:, j * 512 : (j + 1) * 512], hps[:, j], b1gd_t, j)

            o2A = accp.tile([128, 2048], f32, tag="o2A")
            o2B = accp.tile([128, 2048], f32, tag="o2B")
            for jj in range(2):
                ops = pp.tile([128, 4, 16, 32], f32, tag="ps")
                for m in range(2):
                    j = 2 * jj + m
                    nc.tensor.matmul(
                        ops[:, 2 * m],
                        w2gt_t[0:64, :],
                        hR[0:64, j * 512 : (j + 1) * 512],
                    )
                    nc.tensor.matmul(
                        ops[:, 2 * m + 1],
                        w2gt_t[64:128, :],
                        hR[64:128, j * 512 : (j + 1) * 512],
                    )
                    evict(o2A[:, j * 512 : (j + 1) * 512], ops[:, 2 * m],
                          b2g_t, j)
                    evict(o2B[:, j * 512 : (j + 1) * 512], ops[:, 2 * m + 1],
                          b2g_t, j)
                    # outputs: late slices split across two idle queues each
                    # so the final transfers drain in parallel
                    a = outA[:, j * 512 : (j + 1) * 512]
                    av = o2A[:, j * 512 : (j + 1) * 512]
                    b = outB[:, j * 512 : (j + 1) * 512]
                    bv = o2B[:, j * 512 : (j + 1) * 512]
                    if j < 2:
                        nc.sync.dma_start(a, av)
                        nc.scalar.dma_start(b, bv)
                    else:
                        nc.sync.dma_start(a[:, 0:256], av[:, 0:256])
                        nc.gpsimd.dma_start(a[:, 256:512], av[:, 256:512])
                        nc.scalar.dma_start(b[:, 0:256], bv[:, 0:256])
                        nc.gpsimd.dma_start(b[:, 256:512], bv[:, 256:512])

    nc.compile()
    return nc


def _get_program():
    global _PROGRAM
    if _PROGRAM is None:
        _PROGRAM = _build_program()
    return _PROGRAM


def _host_pack(relative_points, W1, b1, W1g, b1g, W2g, b2g):
    from ml_dtypes import bfloat16

    X = np.ascontiguousarray(relative_points, dtype=np.float32)
    W1 = np.asarray(W1, np.float32)
    b1 = np.asarray(b1, np.float32)
    W1g = np.asarray(W1g, np.float32)
    b1g = np.asarray(b1g, np.float32)
    W2g = np.asarray(W2g, np.float32)
    b2g = np.asarray(b2g, np.float32)

    wpack = np.zeros((6, 128), np.float32)
    wpack[0:3, 0:64] = W1.T
    wpack[3:6, 64:128] = W1.T
    wpack = wpack.astype(bfloat16)
    b1d = np.concatenate([b1, b1]).reshape(128, 1)
    w1gs = np.ascontiguousarray(np.vstack([W1g.T, W1g.T])).astype(bfloat16)
    b1gd = np.concatenate([b1g, b1g]).reshape(128, 1)
    w2gt = np.ascontiguousarray(np.vstack([W2g.T, W2g.T])).astype(bfloat16)
    b2gc = np.ascontiguousarray(b2g.reshape(128, 1))

    in_maps = []
    for d in range(NCORES):
        Xc = X[d * NPC : (d + 1) * NPC]
        xt6 = np.ascontiguousarray(
            Xc.reshape(G, 2, 512, 3).transpose(1, 3, 0, 2).reshape(6, G * 512)
        ).astype(bfloat16)
        in_maps.append(
            {
                "xt": xt6,
                "wpack": wpack,
                "b1d": b1d,
                "w1gs": w1gs,
                "b1gd": b1gd,
                "w2gt": w2gt,
                "b2g": b2gc,
            }
        )
    return in_maps


def _host_unpack(results):
    out = np.empty((S, FG1), np.float32)
    for d in range(NCORES):
        oA = results[d]["outA"].reshape(128, NCHUNK, 4, 16)
        oB = results[d]["outB"].reshape(128, NCHUNK, 4, 16)
        blk = out[d * SPC : (d + 1) * SPC].reshape(NCHUNK, 4, 2, 16, 128)
        blk[:, :, 0] = oA.transpose(1, 2, 3, 0)
        blk[:, :, 1] = oB.transpose(1, 2, 3, 0)
    return out


def _numpy_fallback(relative_points, cluster, num_clusters,
                    W1, b1, W1g, b1g, W2g, b2g):
    X = np.asarray(relative_points, np.float32)
    fc1 = np.maximum(X @ np.asarray(W1, np.float32).T + np.asarray(b1, np.float32), 0.0)
    Sn = int(num_clusters)
    cl = np.asarray(cluster).astype(np.int64)
    pooled = np.full((Sn, fc1.shape[1]), -np.inf, np.float32)
    # sorted segment ids -> reduceat over run starts
    starts = np.flatnonzero(np.r_[True, cl[1:] != cl[:-1]])
    seg_ids = cl[starts]
    pooled[seg_ids] = np.maximum.reduceat(fc1, starts, axis=0)
    h = np.maximum(pooled @ np.asarray(W1g, np.float32).T + np.asarray(b1g, np.float32), 0.0)
    return np.maximum(h @ np.asarray(W2g, np.float32).T + np.asarray(b2g, np.float32), 0.0).astype(np.float32)


def _run_hw(in_maps, trace=False):
    from concourse.bass_utils import run_bass_kernel_spmd

    nc = _get_program()
    return run_bass_kernel_spmd(
        nc, in_maps, list(range(NCORES)), trace=trace
    )


def kernel(relative_points, cluster, num_clusters,
           W1, b1, W1g, b1g, W2g, b2g):
    cl = np.asarray(cluster)
    expected_cl = np.arange(N, dtype=np.int64) // PTS_PER_CLUSTER
    if (
        relative_points.shape != (N, 3)
        or int(num_clusters) != S
        or not np.array_equal(cl, expected_cl)
    ):
        return _numpy_fallback(relative_points, cluster, num_clusters,
                               W1, b1, W1g, b1g, W2g, b2g)

    in_maps = _host_pack(relative_points, W1, b1, W1g, b1g, W2g, b2g)
    res = _run_hw(in_maps, trace=False)
    return _host_unpack(res.results)


def run_traced(inputs):
    """test.py helper: returns (output, exec_time_ns)."""
    in_maps = _host_pack(
        inputs["relative_points"], inputs["W1"], inputs["b1"],
        inputs["W1g"], inputs["b1g"], inputs["W2g"], inputs["b2g"],
    )
    res = _run_hw(in_maps, trace=True)
    return _host_unpack(res.results), res.exec_time_ns



# revision 36
# speedup vs baseline: 1.0074x; 1.0074x over previous
"""Trainium2 Bass kernel for OldNeighborhoodEncoder (segment_reduce).

Math (reference):
    fc1    = relu(X @ W1.T + b1)            # [N, 64], X = [N, 3]
    pooled = segment_max(fc1, cluster, S)   # [S, 64], cluster = arange(N)//32
    h      = relu(pooled @ W1g.T + b1g)     # [S, 64]
    out    = relu(h @ W2g.T + b2g)          # [S, 128]

Hardcoded sizes: N=1048576, S=32768 (32 pts/cluster), FEATURE=64, FG0=64,
FG1=128, 8 cores. Data-parallel over points: core d handles points
[d*131072, (d+1)*131072) == clusters [d*4096, (d+1)*4096); no collectives.

Device layout (per core):
  xt [6, 65536]: col c = 512*g + o (g in 0..127, o in 0..511); rows 0-2 =
    xyz of point 1024*g + o, rows 3-5 = xyz of point 1024*g + 512 + o.
  wpack [6,128] = blockdiag(W1.T, W1.T): one matmul column-block computes
    fc1 (pre-bias) for TWO 512-point chunks at once -> full 128-partition
    PE output. Bias+relu are deferred past the max (monotone).
  psum [128,4,16,32]: bank b holds g = 4i+b; view [.., q, t] with o=32q+t,
    so a single DVE reduce over t pools 4*16 = 64 cluster-halves.
  pooled [128, 32, 4, 16]: pooled[64a+f, i, b, q] = max_z of cluster
    128i + 32b + 16a + q, feature f.
  Tail: relu(+b1) -> blockdiag(W1g.T) matmul -> relu(+b1g) ->
    W2g.T matmul (K=64, separately for a=0 from partitions 0:64 and a=1
    from 64:128) -> relu(+b2g) -> outA/outB [128, 2048].

v1.5 perf structure: the main loop is DVE-reduce-bound (Pool/GPSIMD has no
legal max op on this target, so DVE does all 32 chunk reductions); weight
DMAs go on the Scalar queue (HWDGE; gpsimd SWDGE blocked the first matmul
~7us); relu(+b1) of pooled happens in slices during the main loop on ACT;
the tail MLP is pipelined in 512-col sub-slices with relu work split
between ACT and DVE, and output DMAs are split in halves on two queues.
"""

import sys
import numpy as np

if "/opt/trn_rl_repo" not in sys.path:
    sys.path.insert(0, "/opt/trn_rl_repo")

N = 1048576
S = 32768
PTS_PER_CLUSTER = 32
FEATURE = 64
FG0 = 64
FG1 = 128
NCORES = 8
NPC = N // NCORES          # 131072 points per core
SPC = S // NCORES          # 4096 clusters per core
G = NPC // 1024            # 128 column-groups of 512
NCHUNK = 32                # psum chunks per core (each = 4 groups)

_PROGRAM = None  # (nc, input_names) cache


def _build_program():
    from concourse import bacc, bass, tile

    mybir = bass.mybir
    f32 = mybir.dt.float32
    # bf16 matmul path: full-rate 1 cycle/row on PE (f32r measured ~3x
    # slower on hw despite the cost model's claim), halves the xt DMA.
    fmm = mybir.dt.bfloat16
    AX = mybir.AxisListType

    nc = bacc.Bacc("TRN2", target_bir_lowering=False, debug=False)

    xt = nc.dram_tensor("xt", [6, G * 512], fmm, kind="ExternalInput").ap()
    wpack = nc.dram_tensor("wpack", [6, 128], fmm, kind="ExternalInput").ap()
    b1d = nc.dram_tensor("b1d", [128, 1], f32, kind="ExternalInput").ap()
    w1gs = nc.dram_tensor("w1gs", [128, 64], fmm, kind="ExternalInput").ap()
    b1gd = nc.dram_tensor("b1gd", [128, 1], f32, kind="ExternalInput").ap()
    w2gt = nc.dram_tensor("w2gt", [128, 128], fmm, kind="ExternalInput").ap()
    b2g = nc.dram_tensor("b2g", [128, 1], f32, kind="ExternalInput").ap()
    outA = nc.dram_tensor("outA", [128, 2048], f32, kind="ExternalOutput").ap()
    outB = nc.dram_tensor("outB", [128, 2048], f32, kind="ExternalOutput").ap()

    # chunks whose pooling runs as a direct f32 DVE reduce from PSUM; the
    # rest are relu(+b1)-copied PSUM->SBUF bf16 by ACT, then max-pooled on
    # DVE with a tensor_tensor tree (bf16 2x_1p: 2 results/cycle), four
    # chunks per tree pass to amortize the ~95ns/op DVE overhead.
    D_CHUNKS = (2, 5, 8, 11, 14, 17, 20, 23, 26, 29)
    d_index = {k: i for i, k in enumerate(D_CHUNKS)}
    SINGLES = (30, 31)  # last two chunks pool via single-chunk trees so the
    # final pooledR slices land with minimum latency

    Relu = mybir.ActivationFunctionType.Relu
    add = mybir.AluOpType.add
    vmax = mybir.AluOpType.max

    with tile.TileContext(nc) as tc:
        with (
            tc.tile_pool(name="w", bufs=1) as wp,
            tc.tile_pool(name="x", bufs=3) as xp,
            tc.tile_pool(name="pre", bufs=3) as prep,
            tc.tile_pool(name="scr", bufs=2) as scrp,
            tc.tile_pool(name="acc", bufs=1) as accp,
            tc.tile_pool(name="ps", bufs=2, space=bass.MemorySpace.PSUM) as pp,
        ):
            # wpack lives at SBUF partitions 0:6 AND 32:38: consecutive
            # matmuls alternate PE row-groups q0/q1, which the 32x32
            # sub-array hardware runs CONCURRENTLY (2 matmuls per ~427ns
            # instead of ~512ns each -- measured).
            wpack_t = wp.tile([38, 128], fmm, tag="wpack")
            b1d_t = wp.tile([128, 1], f32, tag="b1d")
            w1gs_t = wp.tile([128, 64], fmm, tag="w1gs")
            b1gd_t = wp.tile([128, 1], f32, tag="b1gd")
            w2gt_t = wp.tile([128, 128], fmm, tag="w2gt")
            b2g_t = wp.tile([128, 1], f32, tag="b2g")
            # wpack on the Sync queue ahead of xt: it gates the first
            # matmul, and sync's queue drains it ~1us sooner than scalar's.
            nc.sync.dma_start(wpack_t[0:6, :], wpack[:])
            nc.sync.dma_start(wpack_t[32:38, :], wpack[:])
            for t, d in (
                (b1d_t, b1d),
                (w1gs_t, w1gs),
                (b1gd_t, b1gd),
                (w2gt_t, w2gt),
                (b2g_t, b2g),
            ):
                nc.scalar.dma_start(t[:], d[:])

            pooledF = accp.tile([128, len(D_CHUNKS), 64], f32, tag="pooledF")
            pooledR = accp.tile([128, 2048], fmm, tag="pooledR")
            pooledRv = pooledR[:].rearrange("p (c e) -> p c e", e=64)

            def pair_tree_ops(pt, sc, ks):
                # 5-level pairwise-max tree over two chunks: [128, 2, 64, 32]
                # bf16 -> two pooledR slices [128, 64]. Returns (est_ns,
                # thunk) steps so the flush can be rate-controlled; levels
                # 1-4 run at 2 elem-results/cycle (bf16 2x_1p).
                l1 = sc[:, :, 0:1024].rearrange("p q (g e) -> p q g e", g=64)
                l2 = sc[:, :, 1024:1536].rearrange("p q (g e) -> p q g e", g=64)
                l3 = sc[:, :, 1536:1792].rearrange("p q (g e) -> p q g e", g=64)
                l4 = sc[:, :, 1792:1920].rearrange("p q (g e) -> p q g e", g=64)
                v = pt[:]
                ka, kb = ks
                outv = pooledRv[:, ka : kb + 1 : (kb - ka)]
                tt = nc.vector.tensor_tensor
                return [
                    (1170, lambda: tt(
                        l1, v[:, :, :, 0:16], v[:, :, :, 16:32], op=vmax)),
                    (630, lambda: tt(
                        l2, l1[:, :, :, 0:8], l1[:, :, :, 8:16], op=vmax)),
                    (370, lambda: tt(
                        l3, l2[:, :, :, 0:4], l2[:, :, :, 4:8], op=vmax)),
                    (230, lambda: tt(
                        l4, l3[:, :, :, 0:2], l3[:, :, :, 2:4], op=vmax)),
                    (230, lambda: tt(
                        outv,
                        l4[:, :, :, 0:1].rearrange("p q g e -> p q (g e)"),
                        l4[:, :, :, 1:2].rearrange("p q g e -> p q (g e)"),
                        op=vmax)),
                ]

            def single_tree_ops(pt, sc, k):
                # as pair_tree_ops but for one chunk: [128, 1, 64, 32]
                l1 = sc[:, 0, 0:1024].rearrange("p (g e) -> p g e", g=64)
                l2 = sc[:, 0, 1024:1536].rearrange("p (g e) -> p g e", g=64)
                l3 = sc[:, 0, 1536:1792].rearrange("p (g e) -> p g e", g=64)
                l4 = sc[:, 0, 1792:1920].rearrange("p (g e) -> p g e", g=64)
                v = pt[:, 0]
                outv = pooledR[:, k * 64 : (k + 1) * 64].rearrange(
                    "p (g e) -> p g e", g=64)
                tt = nc.vector.tensor_tensor
                return [
                    (690, lambda: tt(
                        l1, v[:, :, 0:16], v[:, :, 16:32], op=vmax)),
                    (420, lambda: tt(
                        l2, l1[:, :, 0:8], l1[:, :, 8:16], op=vmax)),
                    (290, lambda: tt(
                        l3, l2[:, :, 0:4], l2[:, :, 4:8], op=vmax)),
                    (230, lambda: tt(
                        l4, l3[:, :, 0:2], l3[:, :, 2:4], op=vmax)),
                    (230, lambda: tt(
                        outv, l4[:, :, 0:1], l4[:, :, 1:2], op=vmax)),
                ]

            # main loop: fc1 matmuls + split segment-max pooling. Tree steps
            # are drip-fed into the DVE queue (~<=1.5us per chunk) so a
            # D-chunk's psum-freeing reduce is never stuck behind a long
            # burst of tree work.
            pair = None
            step_q = []
            xt_t = None
            for k in range(NCHUNK):
                if k % 4 == 0:
                    # xt tile: even 512-col blocks at partitions 0:6,
                    # odd blocks at 32:38 (for q0/q1 row-group alternation).
                    # Even halves on the sync queue, odd halves on the (idle)
                    # gpsimd queue — each HWDGE DMA costs ~730ns of queue
                    # issue time, so splitting queues halves the startup
                    # serialization; scalar stays clear for the ACT copies.
                    xt_t = xp.tile([38, 4096], fmm, tag="xt")
                    c0 = k * 2048
                    src = xt[:, c0 : c0 + 8192].rearrange(
                        "p (b c) -> p b c", c=1024)
                    dste = xt_t[0:6, :].rearrange("p (b c) -> p b c", c=512)
                    dsto = xt_t[32:38, :].rearrange("p (b c) -> p b c", c=512)
                    if k == 0:
                        # split so chunk 0's blocks land first
                        nc.sync.dma_start(dste[:, 0:2], src[:, 0:2, 0:512])
                        nc.gpsimd.dma_start(dsto[:, 0:2], src[:, 0:2, 512:1024])
                        nc.sync.dma_start(dste[:, 2:8], src[:, 2:8, 0:512])
                        nc.gpsimd.dma_start(dsto[:, 2:8], src[:, 2:8, 512:1024])
                    else:
                        nc.sync.dma_start(dste[:], src[:, :, 0:512])
                        nc.gpsimd.dma_start(dsto[:], src[:, :, 512:1024])
                ps = pp.tile([128, 4, 16, 32], f32, tag="ps")
                for b in range(4):
                    t = (k % 4) * 4 + b
                    home = 0 if t % 2 == 0 else 32
                    col = (t // 2) * 512
                    nc.tensor.matmul(
                        ps[:, b],
                        wpack_t[home : home + 6, :],
                        xt_t[home : home + 6, col : col + 512],
                    )
                if k in d_index:
                    di = d_index[k]
                    nc.vector.reduce_max(pooledF[:, di], ps[:], axis=AX.X)
                    budget = 420
                elif k in SINGLES:
                    pt = prep.tile([128, 2, 64, 32], fmm, tag="pre")
                    sc = scrp.tile([128, 2, 1920], fmm, tag="scr")
                    nc.scalar.activation(
                        pt[:, 0], ps[:], Relu, bias=b1d_t[:])
                    step_q.extend(single_tree_ops(pt, sc, k))
                    budget = 1500
                else:
                    if pair is None:
                        pt = prep.tile([128, 2, 64, 32], fmm, tag="pre")
                        sc = scrp.tile([128, 2, 1920], fmm, tag="scr")
                        pair = (pt, sc, [])
                    pt, sc, ks = pair
                    nc.scalar.activation(
                        pt[:, len(ks)], ps[:], Relu, bias=b1d_t[:])
                    ks.append(k)
                    if len(ks) == 2:
                        step_q.extend(pair_tree_ops(pt, sc, ks))
                        pair = None
                    budget = 1500
                while step_q and step_q[0][0] <= budget:
                    est, thunk = step_q.pop(0)
                    thunk()
                    budget -= est
            for _, thunk in step_q:
                thunk()
            # relu(+b1) all D-chunk pooled slices in one strided op
            # (D_CHUNKS is an arithmetic sequence, so the output AP is
            # regular: stride D_CHUNKS[1]-D_CHUNKS[0] chunks)
            dstep = D_CHUNKS[1] - D_CHUNKS[0]
            nc.vector.tensor_scalar(
                pooledRv[:, D_CHUNKS[0] : D_CHUNKS[-1] + 1 : dstep],
                pooledF[:], b1d_t[:], 0.0, op0=add, op1=vmax,
            )

            # tail MLP: h = relu(pooledR @ W1g.T + b1g), out = relu(h @ ...).
            # The h matmul splits blockdiag K=128 into two K=64 matmuls on
            # row-groups 0/64 which the PE runs concurrently.
            hps = pp.tile([128, 4, 16, 32], f32, tag="ps")
            hR = accp.tile([128, 2048], fmm, tag="hR")
            def evict(dst, src, bias_t, j):
                # slice 3's evictions ride DVE (clear right after the final
                # tree); everything else goes to ACT, which idles once the
                # loop copies end.
                if j == 3:
                    nc.vector.tensor_scalar(
                        dst, src, bias_t[:], 0.0, op0=add, op1=vmax)
                else:
                    nc.scalar.activation(dst, src, Relu, bias=bias_t[:])

            # slice 3 first everywhere: its chain is the critical path.
            for j in (3, 0, 1, 2):
                nc.tensor.matmul(
                    hps[0:64, j],
                    w1gs_t[0:64, :],
                    pooledR[0:64, j * 512 : (j + 1) * 512],
                )
                nc.tensor.matmul(
                    hps[64:128, j],
                    w1gs_t[64:128, :],
                    pooledR[64:128, j * 512 : (j + 1) * 512],
                )
                evict(hR[:, j * 512 : (j + 1) * 512], hps[:, j], b1gd_t, j)

            o2A = accp.tile([128, 2048], f32, tag="o2A")
            o2B = accp.tile([128, 2048], f32, tag="o2B")
            for jj in range(2):
                ops = pp.tile([128, 4, 16, 32], f32, tag="ps")
                for m in range(2):
                    j = (3, 0, 1, 2)[2 * jj + m]
                    nc.tensor.matmul(
                        ops[:, 2 * m],
                        w2gt_t[0:64, :],
                        hR[0:64, j * 512 : (j + 1) * 512],
                    )
                    nc.tensor.matmul(
                        ops[:, 2 * m + 1],
                        w2gt_t[64:128, :],
                        hR[64:128, j * 512 : (j + 1) * 512],
                    )
                    evict(o2A[:, j * 512 : (j + 1) * 512], ops[:, 2 * m],
                          b2g_t, j)
                    evict(o2B[:, j * 512 : (j + 1) * 512], ops[:, 2 * m + 1],
                          b2g_t, j)
                    # outputs: the last-computed slice splits across two
                    # idle queues so its final transfers drain in parallel
                    a = outA[:, j * 512 : (j + 1) * 512]
                    av = o2A[:, j * 512 : (j + 1) * 512]
                    b = outB[:, j * 512 : (j + 1) * 512]
                    bv = o2B[:, j * 512 : (j + 1) * 512]
                    if j != 3:
                        nc.sync.dma_start(a, av)
                        nc.scalar.dma_start(b, bv)
                    else:
                        nc.sync.dma_start(a[:, 0:256], av[:, 0:256])
                        nc.gpsimd.dma_start(a[:, 256:512], av[:, 256:512])
                        nc.scalar.dma_start(b[:, 0:256], bv[:, 0:256])
                        nc.gpsimd.dma_start(b[:, 256:512], bv[:, 256:512])

    nc.compile()
    return nc


def _get_program():
    global _PROGRAM
    if _PROGRAM is None:
        _PROGRAM = _build_program()
    return _PROGRAM


def _host_pack(relative_points, W1, b1, W1g, b1g, W2g, b2g):
    from ml_dtypes import bfloat16

    X = np.ascontiguousarray(relative_points, dtype=np.float32)
    W1 = np.asarray(W1, np.float32)
    b1 = np.asarray(b1, np.float32)
    W1g = np.asarray(W1g, np.float32)
    b1g = np.asarray(b1g, np.float32)
    W2g = np.asarray(W2g, np.float32)
    b2g = np.asarray(b2g, np.float32)

    wpack = np.zeros((6, 128), np.float32)
    wpack[0:3, 0:64] = W1.T
    wpack[3:6, 64:128] = W1.T
    wpack = wpack.astype(bfloat16)
    b1d = np.concatenate([b1, b1]).reshape(128, 1)
    w1gs = np.ascontiguousarray(np.vstack([W1g.T, W1g.T])).astype(bfloat16)
    b1gd = np.concatenate([b1g, b1g]).reshape(128, 1)
    w2gt = np.ascontiguousarray(np.vstack([W2g.T, W2g.T])).astype(bfloat16)
    b2gc = np.ascontiguousarray(b2g.reshape(128, 1))

    in_maps = []
    for d in range(NCORES):
        Xc = X[d * NPC : (d + 1) * NPC]
        xt6 = np.ascontiguousarray(
            Xc.reshape(G, 2, 512, 3).transpose(1, 3, 0, 2).reshape(6, G * 512)
        ).astype(bfloat16)
        in_maps.append(
            {
                "xt": xt6,
                "wpack": wpack,
                "b1d": b1d,
                "w1gs": w1gs,
                "b1gd": b1gd,
                "w2gt": w2gt,
                "b2g": b2gc,
            }
        )
    return in_maps


def _host_unpack(results):
    out = np.empty((S, FG1), np.float32)
    for d in range(NCORES):
        oA = results[d]["outA"].reshape(128, NCHUNK, 4, 16)
        oB = results[d]["outB"].reshape(128, NCHUNK, 4, 16)
        blk = out[d * SPC : (d + 1) * SPC].reshape(NCHUNK, 4, 2, 16, 128)
        blk[:, :, 0] = oA.transpose(1, 2, 3, 0)
        blk[:, :, 1] = oB.transpose(1, 2, 3, 0)
    return out


def _numpy_fallback(relative_points, cluster, num_clusters,
                    W1, b1, W1g, b1g, W2g, b2g):
    X = np.asarray(relative_points, np.float32)
    fc1 = np.maximum(X @ np.asarray(W1, np.float32).T + np.asarray(b1, np.float32), 0.0)
    Sn = int(num_clusters)
    cl = np.asarray(cluster).astype(np.int64)
    pooled = np.full((Sn, fc1.shape[1]), -np.inf, np.float32)
    # sorted segment ids -> reduceat over run starts
    starts = np.flatnonzero(np.r_[True, cl[1:] != cl[:-1]])
    seg_ids = cl[starts]
    pooled[seg_ids] = np.maximum.reduceat(fc1, starts, axis=0)
    h = np.maximum(pooled @ np.asarray(W1g, np.float32).T + np.asarray(b1g, np.float32), 0.0)
    return np.maximum(h @ np.asarray(W2g, np.float32).T + np.asarray(b2g, np.float32), 0.0).astype(np.float32)


def _run_hw(in_maps, trace=False):
    from concourse.bass_utils import run_bass_kernel_spmd

    nc = _get_program()
    return run_bass_kernel_spmd(
        nc, in_maps, list(range(NCORES)), trace=trace
    )


def kernel(relative_points, cluster, num_clusters,
           W1, b1, W1g, b1g, W2g, b2g):
    cl = np.asarray(cluster)
    expected_cl = np.arange(N, dtype=np.int64) // PTS_PER_CLUSTER
    if (
        relative_points.shape != (N, 3)
        or int(num_clusters) != S
        or not np.array_equal(cl, expected_cl)
    ):
        return _numpy_fallback(relative_points, cluster, num_clusters,
                               W1, b1, W1g, b1g, W2g, b2g)

    in_maps = _host_pack(relative_points, W1, b1, W1g, b1g, W2g, b2g)
    res = _run_hw(in_maps, trace=False)
    return _host_unpack(res.results)


def run_traced(inputs):
    """test.py helper: returns (output, exec_time_ns)."""
    in_maps = _host_pack(
        inputs["relative_points"], inputs["W1"], inputs["b1"],
        inputs["W1g"], inputs["b1g"], inputs["W2g"], inputs["b2g"],
    )
    res = _run_hw(in_maps, trace=True)
    return _host_unpack(res.results), res.exec_time_ns

